# revision 13
# baseline (speedup 1.0000x reference)
"""Trainium2 Bass kernel for nn_AgentGnn_CRAT (2-layer CGConv GNN).

Structure exploited: the graph is B=1024 independent fully-connected
16-agent cliques (no self loops).  For edge (s -> t) within a sample:

    z = [x_t, x_s, c_t - c_s]                       (258 dims)
    m = sigmoid(z @ wf.T + bf) * softplus(z @ ws.T + bs)
    agg[t] = sum_{s != t} m(s, t)
    out = relu(batchnorm(agg) + x)                  (x2 layers)

Since z @ wf.T splits into a target part and a source part,
    a_f(s,t) = P_f[t] + Q_f[s]
      P_f = Wt_f^T x + (Wc_f^T c + bf) = Wt_f^T x + Fc_f
      Q_f = Ws_f^T x - Wc_f^T c        = Ws_f^T x + bf - Fc_f
so the per-edge work is a broadcast add of per-node vectors, done
dense over all 16x16 pairs per sample.  Fc is computed once per node
with the bias folded in via a constant ones-channel appended to the
centers (K=3 matmul), so the inner loop has one matmul per gate.

The compiler's ACT tables have no Softplus; it is computed as
    softplus(b) = ln(exp(b) + 1)
with exp+ln sharing one ACT table (natural_log_exp) and sigmoid its
own, batched TGROUP chunks at a time to amortize table loads.  The
diagonal (s==t) is memset to -30 before sigmoid so those messages
vanish from the aggregation.
    rsqrt(v) = exp(-0.5 * ln(v))          (for batchnorm)

Sharding: data parallel over samples -- each of 8 cores gets 2048
nodes (128 samples).  BatchNorm batch stats are combined with a tiny
[128,3] AllReduce (mean, var, mean^2 per feature); a dummy warm-up
AllReduce at kernel start absorbs the collective's first-use cost.

Layout: features (H=128) on partitions, nodes/pairs on the free axis;
bf16 pair stage, fp32 P/Q/aggregation/batchnorm.  Host pre-transposes
inputs and post-transposes the output.
"""

import numpy as np

H = 128          # latent dim = partition dim
D = 2            # edge attr dim
A = 16           # agents per sample
B = 1024         # samples
N = B * A        # 16384 nodes
N_CORES = 8
NL = N // N_CORES        # 2048 nodes per core
SL = NL // A             # 128 samples per core
CS = 8                   # samples per phase-2 chunk
PC = CS * A * A          # 2048 pair columns per chunk
NCH = SL // CS           # 16 chunks
BLK = 512                # matmul free-dim block (one PSUM bank of f32)
EPS = 1e-5
DIAG_KILL = -30.0        # sigmoid(-30) ~ 1e-13 -> diagonal message ~ 0
TGROUP = 4               # chunks per ACT-table batch (amortizes table loads)
A_POOL_CHUNKS = (3, 7, 11, 15)   # chunks whose a-add runs on Pool

_CACHE = {}


# --------------------------------------------------------------------------
# bass program
# --------------------------------------------------------------------------

def _build_bass():
    from concourse import bass, bacc, tile, mybir

    f32 = mybir.dt.float32
    bf16 = mybir.dt.bfloat16
    AF = mybir.ActivationFunctionType
    OP = mybir.AluOpType

    nc = bacc.Bacc("TRN2", target_bir_lowering=False, debug=False,
                   num_devices=N_CORES)

    xT = nc.dram_tensor("xT", [H, NL], f32, kind="ExternalInput").ap()
    # centers + ones channel: rows (c0, c1, 1)
    cA = nc.dram_tensor("cA", [D + 1, NL], bf16, kind="ExternalInput").ap()
    # 8 blocks of [128,128] lhsT weights: per layer (wt_f, ws_f, wt_s, ws_s)
    Wd = nc.dram_tensor("W", [H, 8 * H], f32, kind="ExternalInput").ap()
    # 4 blocks of [3,128]: per layer (wc_f|bf, wc_s|bs)
    WCd = nc.dram_tensor("WC", [D + 1, 4 * H], bf16,
                         kind="ExternalInput").ap()
    # per-feature vectors: cols = (bf, bs, gamma, beta) x 2 layers
    Vd = nc.dram_tensor("V", [H, 8], f32, kind="ExternalInput").ap()
    outT = nc.dram_tensor("outT", [H, NL], f32, kind="ExternalOutput").ap()

    with tile.TileContext(nc) as tc:
        with (
            tc.tile_pool(name="res", bufs=1) as res,
            tc.tile_pool(name="pq", bufs=1) as pqp,
            tc.tile_pool(name="ch", bufs=3) as ch,
            tc.tile_pool(name="act", bufs=TGROUP) as chact,
            tc.tile_pool(name="psum", bufs=2, space="PSUM") as psp,
            tc.tile_pool(name="dram", bufs=1, space="DRAM") as dram,
        ):
            x0 = res.tile([H, NL], f32, tag="x0", name="x0")
            ca = res.tile([D + 1, NL], bf16, tag="ca", name="ca")
            w = res.tile([H, 8 * H], f32, tag="w", name="w")
            wca = res.tile([D + 1, 4 * H], bf16, tag="wca", name="wca")
            v = res.tile([H, 8], f32, tag="v", name="v")
            nc.sync.dma_start(w[:, 0:4 * H], Wd[:, 0:4 * H])
            nc.sync.dma_start(wca[:], WCd[:])
            nc.sync.dma_start(ca[:], cA[:])
            nc.sync.dma_start(v[:], Vd[:])
            for blk in range(NL // BLK):
                sl = slice(blk * BLK, (blk + 1) * BLK)
                nc.sync.dma_start(x0[:, sl], xT[:, sl])
            nc.sync.dma_start(w[:, 4 * H:8 * H], Wd[:, 4 * H:8 * H])

            # dummy collective: absorbs first-collective setup latency
            # concurrently with phase 1 instead of on the critical path
            wdi = dram.tile([H, 1], f32, tag="wdi", name="wdi")
            wdo = dram.tile([H, 1], f32, tag="wdo", name="wdo")
            wds = res.tile([H, 1], f32, tag="wds", name="wds")
            nc.gpsimd.memset(wds[:], 0.0)
            nc.sync.dma_start(wdi[:], wds[:])
            nc.gpsimd.collective_compute(
                "AllReduce", OP.add, ins=[wdi.opt()], outs=[wdo.opt()],
                replica_groups=[list(range(N_CORES))])

            # Fc = wc^T c + b for all gates/layers up front (PE is
            # otherwise idle during layer-1 chunks; bias rides the ones
            # channel of cA)
            Fcs_all = {}
            for l in range(2):
                for g in range(2):
                    Fc = pqp.tile([H, NL], f32, tag=f"Fc{l}{g}",
                                  name=f"Fc{l}{g}")
                    Fcs_all[(l, g)] = Fc
                    cb = (l * 2 + g) * H
                    for blk in range(NL // BLK):
                        sl = slice(blk * BLK, (blk + 1) * BLK)
                        psc = psp.tile([H, BLK], f32, tag="psC",
                                       name=f"psC{l}_{blk}_{g}")
                        nc.tensor.matmul(psc[:], wca[:, cb:cb + H], ca[:, sl],
                                         start=True, stop=True)
                        nc.scalar.activation(Fc[:, sl], psc[:], AF.Identity)

            x_in = x0
            for l in range(2):
                # ------------- phase 1: per-node P/Q matmuls -------------
                Pf = pqp.tile([H, NL], f32, tag="Pf", name=f"Pf{l}")
                Qf = pqp.tile([H, NL], f32, tag="Qf", name=f"Qf{l}")
                Ps = pqp.tile([H, NL], f32, tag="Ps", name=f"Ps{l}")
                Qs = pqp.tile([H, NL], f32, tag="Qs", name=f"Qs{l}")
                for blk in range(NL // BLK):
                    sl = slice(blk * BLK, (blk + 1) * BLK)
                    for g, (Pt, Qt) in enumerate(((Pf, Qf), (Ps, Qs))):
                        Fc = Fcs_all[(l, g)]
                        wb = l * 4 * H + g * 2 * H
                        bias = v[:, l * 4 + g:l * 4 + g + 1]
                        # P = wt^T x + Fc
                        ps1 = psp.tile([H, BLK], f32, tag="psP",
                                       name=f"psP{l}_{blk}_{g}")
                        nc.tensor.matmul(ps1[:], w[:, wb:wb + H], x_in[:, sl],
                                         start=True, stop=True)
                        nc.vector.tensor_tensor(Pt[:, sl], ps1[:], Fc[:, sl],
                                                op=OP.add)
                        # Q = ws^T x + b - Fc
                        ps2 = psp.tile([H, BLK], f32, tag="psQ",
                                       name=f"psQ{l}_{blk}_{g}")
                        nc.tensor.matmul(ps2[:], w[:, wb + H:wb + 2 * H],
                                         x_in[:, sl], start=True, stop=True)
                        nc.vector.scalar_tensor_tensor(
                            Qt[:, sl], ps2[:], bias, Fc[:, sl],
                            op0=OP.add, op1=OP.subtract)

                # ------------- phase 2: pair stage -----------------------
                agg = pqp.tile([H, NL], f32, tag="agg", name=f"agg{l}")
                stats = res.tile([H, NCH * 6], f32, tag="stats",
                                 name=f"stats{l}")

                def pair_view(src, ci, is_target):
                    ncols = slice(ci * CS * A, (ci + 1) * CS * A)
                    return (src[:, ncols]
                            .rearrange("p (b t) -> p b t", b=CS)
                            .unsqueeze(3 if is_target else 2)
                            .broadcast_to([H, CS, A, A]))

                for cg in range(NCH // TGROUP):
                    group = [cg * TGROUP + k for k in range(TGROUP)]
                    a2s, bts, Gs, Us = {}, {}, {}, {}
                    for ci in group:
                        a2 = ch.tile([H, PC], bf16, tag="a2",
                                     name=f"a2_{l}_{ci}")
                        a2s[ci] = a2
                        a24 = a2[:].rearrange("p (b t s) -> p b t s",
                                              b=CS, t=A)
                        eng = nc.gpsimd if ci in A_POOL_CHUNKS else nc.vector
                        eng.tensor_tensor(a24, pair_view(Pf, ci, True),
                                          pair_view(Qf, ci, False),
                                          op=OP.add)
                        # kill diagonal (s==t): sigmoid -> ~0
                        diag = (a2[:].rearrange("p (b q) -> p b q", b=CS)
                                [:, :, 0:A * A:A + 1])
                        nc.gpsimd.memset(diag, DIAG_KILL)
                        bt = ch.tile([H, PC], bf16, tag="bt",
                                     name=f"bt_{l}_{ci}")
                        bts[ci] = bt
                        bt4 = bt[:].rearrange("p (b t s) -> p b t s",
                                              b=CS, t=A)
                        nc.gpsimd.tensor_tensor(bt4, pair_view(Ps, ci, True),
                                                pair_view(Qs, ci, False),
                                                op=OP.add)
                    # table A (sigmoid_and_others), batched over the group
                    for ci in group:
                        G = chact.tile([H, PC], bf16, tag="G",
                                       name=f"G_{l}_{ci}")
                        Gs[ci] = G
                        nc.scalar.activation(G[:], a2s[ci][:], AF.Sigmoid)
                    # table B (natural_log_exp): exp then softplus ln
                    for ci in group:
                        U = chact.tile([H, PC], bf16, tag="U",
                                       name=f"U_{l}_{ci}")
                        Us[ci] = U
                        nc.scalar.activation(U[:], bts[ci][:], AF.Exp)
                    for ci in group:
                        ncols = slice(ci * CS * A, (ci + 1) * CS * A)
                        # softplus in place over U, then m = G * sp over U
                        nc.scalar.activation(Us[ci][:], Us[ci][:], AF.Ln,
                                             bias=1.0)
                        m = Us[ci][:]
                        nc.vector.tensor_tensor(m, Gs[ci][:], m,
                                                op=OP.mult)
                        nc.vector.tensor_reduce(
                            agg[:, ncols],
                            m.rearrange("p (n s) -> p n s", s=A),
                            axis=mybir.AxisListType.X, op=OP.add)
                        nc.vector.bn_stats(stats[:, ci * 6:(ci + 1) * 6],
                                           agg[:, ncols])

                # ------------- phase 3: BN + residual + relu -------------
                pack = res.tile([H, 4], f32, tag="pack", name=f"pack{l}")
                nc.vector.bn_aggr(pack[:, 0:2], stats[:])
                nc.scalar.activation(pack[:, 2:3], pack[:, 0:1], AF.Square)

                cin = dram.tile([H, 3], f32, tag=f"cin{l}", name=f"cin{l}")
                cout = dram.tile([H, 3], f32, tag=f"cout{l}", name=f"cout{l}")
                nc.sync.dma_start(cin[:], pack[:, 0:3])
                nc.gpsimd.collective_compute(
                    "AllReduce", OP.add,
                    ins=[cin.opt()], outs=[cout.opt()],
                    replica_groups=[list(range(N_CORES))])
                red = res.tile([H, 3], f32, tag="red", name=f"red{l}")
                nc.sync.dma_start(red[:], cout[:])

                bnp = res.tile([H, 12], f32, tag="bnp", name=f"bnp{l}")
                (mg, ex2t, ex2, msq, var, vare, lnv, inv, sca, tb,
                 bia) = (bnp[:, i:i + 1] for i in range(11))
                nc.vector.tensor_scalar_mul(mg, red[:, 0:1], 1.0 / N_CORES)
                nc.vector.tensor_tensor(ex2t, red[:, 1:2], red[:, 2:3],
                                        op=OP.add)
                nc.vector.tensor_scalar_mul(ex2, ex2t, 1.0 / N_CORES)
                nc.vector.tensor_tensor(msq, mg, mg, op=OP.mult)
                nc.vector.tensor_tensor(var, ex2, msq, op=OP.subtract)
                nc.vector.tensor_scalar_add(vare, var, EPS)
                # rsqrt via the exp/ln table: exp(-0.5 * ln(v))
                nc.scalar.activation(lnv, vare, AF.Ln)
                nc.scalar.activation(inv, lnv, AF.Exp, scale=-0.5)
                nc.vector.tensor_tensor(sca, inv, v[:, l * 4 + 2:l * 4 + 3],
                                        op=OP.mult)
                nc.vector.tensor_tensor(tb, mg, sca, op=OP.mult)
                nc.vector.tensor_tensor(bia, v[:, l * 4 + 3:l * 4 + 4], tb,
                                        op=OP.subtract)

                # y = agg*sca + x, in place over agg; out = relu(y + bia)
                # blocked so layer-2 matmuls / the output DMA start early
                if l == 0:
                    xn = res.tile([H, NL], f32, tag="x1", name="x1")
                else:
                    xn = res.tile([H, NL], f32, tag="xout", name="xout")
                for blk in range(NL // BLK):
                    sl = slice(blk * BLK, (blk + 1) * BLK)
                    nc.vector.scalar_tensor_tensor(
                        agg[:, sl], agg[:, sl], sca, x_in[:, sl],
                        op0=OP.mult, op1=OP.add)
                    nc.scalar.activation(xn[:, sl], agg[:, sl], AF.Relu,
                                         bias=bia)
                    if l == 1:
                        nc.sync.dma_start(outT[:, sl], xn[:, sl])
                x_in = xn

    nc.compile()
    return nc


def get_nc():
    if "nc" not in _CACHE:
        _CACHE["nc"] = _build_bass()
    return _CACHE["nc"]


# --------------------------------------------------------------------------
# host-side sharding / packing
# --------------------------------------------------------------------------

def prep_in_maps(gnn_in, centers, wf1, bf1, ws1, bs1, g1, be1,
                 wf2, bf2, ws2, bs2, g2, be2):
    import ml_dtypes
    bfd = ml_dtypes.bfloat16
    blocks_w, blocks_wc, cols_v = [], [], []
    for wf_, bf_, ws_, bs_, gm_, be_ in ((wf1, bf1, ws1, bs1, g1, be1),
                                         (wf2, bf2, ws2, bs2, g2, be2)):
        for mat, b_ in ((wf_, bf_), (ws_, bs_)):
            blocks_w.append(mat[:, :H].T)                  # wt
            blocks_w.append(mat[:, H:2 * H].T)             # ws
            # [3,128] = (wc0, wc1, bias)
            blocks_wc.append(np.concatenate(
                [mat[:, 2 * H:2 * H + D].T, b_[None, :]], axis=0))
        cols_v += [bf_, bs_, gm_, be_]
    W = np.ascontiguousarray(np.concatenate(blocks_w, axis=1),
                             dtype=np.float32)             # [128,1024]
    WC = np.ascontiguousarray(np.concatenate(blocks_wc, axis=1)).astype(bfd)
    V = np.ascontiguousarray(np.stack(cols_v, axis=1), dtype=np.float32)

    in_maps = []
    for cid in range(N_CORES):
        rows = slice(cid * NL, (cid + 1) * NL)
        cx = centers[rows].T                               # [2, NL]
        ca = np.concatenate([cx, np.ones((1, NL), np.float32)], axis=0)
        in_maps.append({
            "xT": np.ascontiguousarray(gnn_in[rows].T, dtype=np.float32),
            "cA": np.ascontiguousarray(ca).astype(bfd),
            "W": W, "WC": WC, "V": V,
        })
    return in_maps


def _canonical_edge_index():
    i, j = np.meshgrid(np.arange(A), np.arange(A), indexing="ij")
    mask = i != j
    li, lj = i[mask], j[mask]
    offs = (np.arange(B) * A)[:, None]
    rows = (li[None, :] + offs).reshape(-1)
    cols = (lj[None, :] + offs).reshape(-1)
    return np.stack([rows, cols])


def _numpy_fallback(gnn_in, centers, edge_index, params):
    """Generic (slow) host implementation for non-canonical edge_index."""
    row, col = np.asarray(edge_index[0]), np.asarray(edge_index[1])
    eattr = centers[col] - centers[row]
    x = gnn_in

    def softplus(z):
        return np.maximum(z, 0.0) + np.log1p(np.exp(-np.abs(z)))

    def cgconv(x, wf, bf, ws, bs, gm, be):
        z = np.concatenate([x[col], x[row], eattr], axis=-1)
        mf = 1.0 / (1.0 + np.exp(-(z @ wf.T + bf)))
        m = mf * softplus(z @ ws.T + bs)
        agg = np.zeros_like(x)
        np.add.at(agg, col, m)
        mean = agg.mean(axis=0)
        var = agg.var(axis=0)
        bn = (agg - mean) / np.sqrt(var + EPS) * gm + be
        return bn + x

    x = np.maximum(cgconv(x, *params[0]), 0.0)
    x = np.maximum(cgconv(x, *params[1]), 0.0)
    return x.astype(np.float32)


# --------------------------------------------------------------------------
# entry point
# --------------------------------------------------------------------------

def kernel(gnn_in, centers, edge_index, wf1, bf1, ws1, bs1, g1, be1,
           wf2, bf2, ws2, bs2, g2, be2):
    gnn_in = np.asarray(gnn_in, dtype=np.float32)
    centers = np.asarray(centers, dtype=np.float32)
    args = [np.asarray(a, dtype=np.float32)
            for a in (wf1, bf1, ws1, bs1, g1, be1,
                      wf2, bf2, ws2, bs2, g2, be2)]

    ei = np.asarray(edge_index)
    if ei.shape != (2, B * A * (A - 1)) or \
            not np.array_equal(ei, _canonical_edge_index()):
        return _numpy_fallback(gnn_in, centers, ei,
                               (tuple(args[0:6]), tuple(args[6:12])))

    from concourse import bass_utils
    nc = get_nc()
    in_maps = prep_in_maps(gnn_in, centers, *args)
    res = bass_utils.run_bass_kernel_spmd(nc, in_maps,
                                          core_ids=list(range(N_CORES)))
    out = np.empty((N, H), dtype=np.float32)
    for cid in range(N_CORES):
        out[cid * NL:(cid + 1) * NL] = res.results[cid]["outT"].T
    return out


# revision 15
# speedup vs baseline: 1.0138x; 1.0138x over previous
"""Trainium2 Bass kernel for nn_AgentGnn_CRAT (2-layer CGConv GNN).

Structure exploited: the graph is B=1024 independent fully-connected
16-agent cliques (no self loops).  For edge (s -> t) within a sample:

    z = [x_t, x_s, c_t - c_s]                       (258 dims)
    m = sigmoid(z @ wf.T + bf) * softplus(z @ ws.T + bs)
    agg[t] = sum_{s != t} m(s, t)
    out = relu(batchnorm(agg) + x)                  (x2 layers)

Since z @ wf.T splits into a target part and a source part,
    a_f(s,t) = P_f[t] + Q_f[s]
      P_f = Wt_f^T x + (Wc_f^T c + bf) = Wt_f^T x + Fc_f
      Q_f = Ws_f^T x - Wc_f^T c        = Ws_f^T x + bf - Fc_f
so the per-edge work is a broadcast add of per-node vectors, done
dense over all 16x16 pairs per sample.  Fc is computed once per node
with the bias folded in via a constant ones-channel appended to the
centers (K=3 matmul), so the inner loop has one matmul per gate.

The compiler's ACT tables have no Softplus; it is computed as
    softplus(b) = ln(exp(b) + 1)
with exp+ln sharing one ACT table (natural_log_exp) and sigmoid its
own, batched TGROUP chunks at a time to amortize table loads.  The
diagonal (s==t) is memset to -30 before sigmoid so those messages
vanish from the aggregation.
    rsqrt(v) = exp(-0.5 * ln(v))          (for batchnorm)

Sharding: data parallel over samples -- each of 8 cores gets 2048
nodes (128 samples).  BatchNorm batch stats are combined with a tiny
[128,3] AllReduce (mean, var, mean^2 per feature); a dummy warm-up
AllReduce at kernel start absorbs the collective's first-use cost.

Layout: features (H=128) on partitions, nodes/pairs on the free axis;
bf16 pair stage, fp32 P/Q/aggregation/batchnorm.  Host pre-transposes
inputs and post-transposes the output.
"""

import numpy as np

H = 128          # latent dim = partition dim
D = 2            # edge attr dim
A = 16           # agents per sample
B = 1024         # samples
N = B * A        # 16384 nodes
N_CORES = 8
NL = N // N_CORES        # 2048 nodes per core
SL = NL // A             # 128 samples per core
CS = 8                   # samples per phase-2 chunk
PC = CS * A * A          # 2048 pair columns per chunk
NCH = SL // CS           # 16 chunks
BLK = 512                # matmul free-dim block (one PSUM bank of f32)
EPS = 1e-5
DIAG_KILL = -30.0        # sigmoid(-30) ~ 1e-13 -> diagonal message ~ 0
TGROUP = 4               # chunks per ACT-table batch (amortizes table loads)
A_POOL_CHUNKS = (3, 6, 9, 12, 15)   # chunks whose a-add runs on Pool

_CACHE = {}


# --------------------------------------------------------------------------
# bass program
# --------------------------------------------------------------------------

def _build_bass():
    from concourse import bass, bacc, tile, mybir

    f32 = mybir.dt.float32
    bf16 = mybir.dt.bfloat16
    AF = mybir.ActivationFunctionType
    OP = mybir.AluOpType

    nc = bacc.Bacc("TRN2", target_bir_lowering=False, debug=False,
                   num_devices=N_CORES)

    xT = nc.dram_tensor("xT", [H, NL], f32, kind="ExternalInput").ap()
    # centers + ones channel: rows (c0, c1, 1)
    cA = nc.dram_tensor("cA", [D + 1, NL], bf16, kind="ExternalInput").ap()
    # 8 blocks of [128,128] lhsT weights: per layer (wt_f, ws_f, wt_s, ws_s)
    Wd = nc.dram_tensor("W", [H, 8 * H], f32, kind="ExternalInput").ap()
    # 4 blocks of [3,128]: per layer (wc_f|bf, wc_s|bs)
    WCd = nc.dram_tensor("WC", [D + 1, 4 * H], bf16,
                         kind="ExternalInput").ap()
    # per-feature vectors: cols = (bf, bs, gamma, beta) x 2 layers
    Vd = nc.dram_tensor("V", [H, 8], f32, kind="ExternalInput").ap()
    outT = nc.dram_tensor("outT", [H, NL], f32, kind="ExternalOutput").ap()

    with tile.TileContext(nc) as tc:
        with (
            tc.tile_pool(name="res", bufs=1) as res,
            tc.tile_pool(name="pq", bufs=1) as pqp,
            tc.tile_pool(name="ch", bufs=3) as ch,
            tc.tile_pool(name="act", bufs=TGROUP) as chact,
            tc.tile_pool(name="psum", bufs=2, space="PSUM") as psp,
            tc.tile_pool(name="dram", bufs=1, space="DRAM") as dram,
        ):
            x0 = res.tile([H, NL], f32, tag="x0", name="x0")
            ca = res.tile([D + 1, NL], bf16, tag="ca", name="ca")
            w = res.tile([H, 8 * H], f32, tag="w", name="w")
            wca = res.tile([D + 1, 4 * H], bf16, tag="wca", name="wca")
            v = res.tile([H, 8], f32, tag="v", name="v")
            nc.sync.dma_start(wca[:], WCd[:])
            nc.sync.dma_start(ca[:], cA[:])
            nc.sync.dma_start(v[:], Vd[:])
            nc.sync.dma_start(w[:, 0:4 * H], Wd[:, 0:4 * H])
            for blk in range(NL // BLK):
                sl = slice(blk * BLK, (blk + 1) * BLK)
                nc.sync.dma_start(x0[:, sl], xT[:, sl])
            nc.sync.dma_start(w[:, 4 * H:8 * H], Wd[:, 4 * H:8 * H])

            # dummy collective: absorbs first-collective setup latency
            # concurrently with phase 1 instead of on the critical path
            wdi = dram.tile([H, 1], f32, tag="wdi", name="wdi")
            wdo = dram.tile([H, 1], f32, tag="wdo", name="wdo")
            wds = res.tile([H, 1], f32, tag="wds", name="wds")
            nc.gpsimd.memset(wds[:], 0.0)
            nc.sync.dma_start(wdi[:], wds[:])
            nc.gpsimd.collective_compute(
                "AllReduce", OP.add, ins=[wdi.opt()], outs=[wdo.opt()],
                replica_groups=[list(range(N_CORES))])

            # Fc = wc^T c + b for all gates/layers up front (PE is
            # otherwise idle during layer-1 chunks; bias rides the ones
            # channel of cA)
            Fcs_all = {}
            for l in range(2):
                for g in range(2):
                    Fc = pqp.tile([H, NL], f32, tag=f"Fc{l}{g}",
                                  name=f"Fc{l}{g}")
                    Fcs_all[(l, g)] = Fc
                    cb = (l * 2 + g) * H
                    for blk in range(NL // BLK):
                        sl = slice(blk * BLK, (blk + 1) * BLK)
                        psc = psp.tile([H, BLK], f32, tag="psC",
                                       name=f"psC{l}_{blk}_{g}")
                        nc.tensor.matmul(psc[:], wca[:, cb:cb + H], ca[:, sl],
                                         start=True, stop=True)
                        nc.scalar.activation(Fc[:, sl], psc[:], AF.Identity)

            x_in = x0
            for l in range(2):
                # ------------- phase 1: per-node P/Q matmuls -------------
                Pf = pqp.tile([H, NL], f32, tag="Pf", name=f"Pf{l}")
                Qf = pqp.tile([H, NL], f32, tag="Qf", name=f"Qf{l}")
                Ps = pqp.tile([H, NL], f32, tag="Ps", name=f"Ps{l}")
                Qs = pqp.tile([H, NL], f32, tag="Qs", name=f"Qs{l}")
                for blk in range(NL // BLK):
                    sl = slice(blk * BLK, (blk + 1) * BLK)
                    for g, (Pt, Qt) in enumerate(((Pf, Qf), (Ps, Qs))):
                        Fc = Fcs_all[(l, g)]
                        wb = l * 4 * H + g * 2 * H
                        bias = v[:, l * 4 + g:l * 4 + g + 1]
                        # P = wt^T x + Fc
                        ps1 = psp.tile([H, BLK], f32, tag="psP",
                                       name=f"psP{l}_{blk}_{g}")
                        nc.tensor.matmul(ps1[:], w[:, wb:wb + H], x_in[:, sl],
                                         start=True, stop=True)
                        nc.vector.tensor_tensor(Pt[:, sl], ps1[:], Fc[:, sl],
                                                op=OP.add)
                        # Q = ws^T x + b - Fc
                        ps2 = psp.tile([H, BLK], f32, tag="psQ",
                                       name=f"psQ{l}_{blk}_{g}")
                        nc.tensor.matmul(ps2[:], w[:, wb + H:wb + 2 * H],
                                         x_in[:, sl], start=True, stop=True)
                        nc.vector.scalar_tensor_tensor(
                            Qt[:, sl], ps2[:], bias, Fc[:, sl],
                            op0=OP.add, op1=OP.subtract)

                # ------------- phase 2: pair stage -----------------------
                agg = pqp.tile([H, NL], f32, tag="agg", name=f"agg{l}")
                stats = res.tile([H, NCH * 6], f32, tag="stats",
                                 name=f"stats{l}")

                def pair_view(src, ci, is_target):
                    ncols = slice(ci * CS * A, (ci + 1) * CS * A)
                    return (src[:, ncols]
                            .rearrange("p (b t) -> p b t", b=CS)
                            .unsqueeze(3 if is_target else 2)
                            .broadcast_to([H, CS, A, A]))

                for cg in range(NCH // TGROUP):
                    group = [cg * TGROUP + k for k in range(TGROUP)]
                    a2s, bts, Gs, Us = {}, {}, {}, {}
                    for ci in group:
                        a2 = ch.tile([H, PC], bf16, tag="a2",
                                     name=f"a2_{l}_{ci}")
                        a2s[ci] = a2
                        a24 = a2[:].rearrange("p (b t s) -> p b t s",
                                              b=CS, t=A)
                        eng = nc.gpsimd if ci in A_POOL_CHUNKS else nc.vector
                        eng.tensor_tensor(a24, pair_view(Pf, ci, True),
                                          pair_view(Qf, ci, False),
                                          op=OP.add)
                        # kill diagonal (s==t): sigmoid -> ~0
                        diag = (a2[:].rearrange("p (b q) -> p b q", b=CS)
                                [:, :, 0:A * A:A + 1])
                        nc.gpsimd.memset(diag, DIAG_KILL)
                        bt = ch.tile([H, PC], bf16, tag="bt",
                                     name=f"bt_{l}_{ci}")
                        bts[ci] = bt
                        bt4 = bt[:].rearrange("p (b t s) -> p b t s",
                                              b=CS, t=A)
                        nc.gpsimd.tensor_tensor(bt4, pair_view(Ps, ci, True),
                                                pair_view(Qs, ci, False),
                                                op=OP.add)
                    # table A (sigmoid_and_others), batched over the group
                    for ci in group:
                        G = chact.tile([H, PC], bf16, tag="G",
                                       name=f"G_{l}_{ci}")
                        Gs[ci] = G
                        nc.scalar.activation(G[:], a2s[ci][:], AF.Sigmoid)
                    # table B (natural_log_exp): exp then softplus ln
                    for ci in group:
                        U = chact.tile([H, PC], bf16, tag="U",
                                       name=f"U_{l}_{ci}")
                        Us[ci] = U
                        nc.scalar.activation(U[:], bts[ci][:], AF.Exp)
                    for ci in group:
                        ncols = slice(ci * CS * A, (ci + 1) * CS * A)
                        # softplus in place over U, then m = G * sp over U
                        nc.scalar.activation(Us[ci][:], Us[ci][:], AF.Ln,
                                             bias=1.0)
                        m = Us[ci][:]
                        nc.vector.tensor_tensor(m, Gs[ci][:], m,
                                                op=OP.mult)
                        nc.vector.tensor_reduce(
                            agg[:, ncols],
                            m.rearrange("p (n s) -> p n s", s=A),
                            axis=mybir.AxisListType.X, op=OP.add)
                        nc.vector.bn_stats(stats[:, ci * 6:(ci + 1) * 6],
                                           agg[:, ncols])

                # ------------- phase 3: BN + residual + relu -------------
                pack = res.tile([H, 4], f32, tag="pack", name=f"pack{l}")
                nc.vector.bn_aggr(pack[:, 0:2], stats[:])
                nc.scalar.activation(pack[:, 2:3], pack[:, 0:1], AF.Square)

                cin = dram.tile([H, 3], f32, tag=f"cin{l}", name=f"cin{l}")
                cout = dram.tile([H, 3], f32, tag=f"cout{l}", name=f"cout{l}")
                nc.sync.dma_start(cin[:], pack[:, 0:3])
                nc.gpsimd.collective_compute(
                    "AllReduce", OP.add,
                    ins=[cin.opt()], outs=[cout.opt()],
                    replica_groups=[list(range(N_CORES))])
                red = res.tile([H, 3], f32, tag="red", name=f"red{l}")
                nc.sync.dma_start(red[:], cout[:])

                bnp = res.tile([H, 12], f32, tag="bnp", name=f"bnp{l}")
                (mg, ex2t, ex2, msq, var, vare, lnv, inv, sca, tb,
                 bia) = (bnp[:, i:i + 1] for i in range(11))
                nc.vector.tensor_scalar_mul(mg, red[:, 0:1], 1.0 / N_CORES)
                nc.vector.tensor_tensor(ex2t, red[:, 1:2], red[:, 2:3],
                                        op=OP.add)
                nc.vector.tensor_scalar_mul(ex2, ex2t, 1.0 / N_CORES)
                nc.vector.tensor_tensor(msq, mg, mg, op=OP.mult)
                nc.vector.tensor_tensor(var, ex2, msq, op=OP.subtract)
                nc.vector.tensor_scalar_add(vare, var, EPS)
                # rsqrt via the exp/ln table: exp(-0.5 * ln(v))
                nc.scalar.activation(lnv, vare, AF.Ln)
                nc.scalar.activation(inv, lnv, AF.Exp, scale=-0.5)
                nc.vector.tensor_tensor(sca, inv, v[:, l * 4 + 2:l * 4 + 3],
                                        op=OP.mult)
                nc.vector.tensor_tensor(tb, mg, sca, op=OP.mult)
                nc.vector.tensor_tensor(bia, v[:, l * 4 + 3:l * 4 + 4], tb,
                                        op=OP.subtract)

                # y = agg*sca + x, in place over agg; out = relu(y + bia)
                # blocked so layer-2 matmuls / the output DMA start early
                if l == 0:
                    xn = res.tile([H, NL], f32, tag="x1", name="x1")
                else:
                    xn = res.tile([H, NL], f32, tag="xout", name="xout")
                for blk in range(NL // BLK):
                    sl = slice(blk * BLK, (blk + 1) * BLK)
                    nc.vector.scalar_tensor_tensor(
                        agg[:, sl], agg[:, sl], sca, x_in[:, sl],
                        op0=OP.mult, op1=OP.add)
                    nc.scalar.activation(xn[:, sl], agg[:, sl], AF.Relu,
                                         bias=bia)
                    if l == 1:
                        nc.sync.dma_start(outT[:, sl], xn[:, sl])
                x_in = xn

    nc.compile()
    return nc


def get_nc():
    if "nc" not in _CACHE:
        _CACHE["nc"] = _build_bass()
    return _CACHE["nc"]


# --------------------------------------------------------------------------
# host-side sharding / packing
# --------------------------------------------------------------------------

def prep_in_maps(gnn_in, centers, wf1, bf1, ws1, bs1, g1, be1,
                 wf2, bf2, ws2, bs2, g2, be2):
    import ml_dtypes
    bfd = ml_dtypes.bfloat16
    blocks_w, blocks_wc, cols_v = [], [], []
    for wf_, bf_, ws_, bs_, gm_, be_ in ((wf1, bf1, ws1, bs1, g1, be1),
                                         (wf2, bf2, ws2, bs2, g2, be2)):
        for mat, b_ in ((wf_, bf_), (ws_, bs_)):
            blocks_w.append(mat[:, :H].T)                  # wt
            blocks_w.append(mat[:, H:2 * H].T)             # ws
            # [3,128] = (wc0, wc1, bias)
            blocks_wc.append(np.concatenate(
                [mat[:, 2 * H:2 * H + D].T, b_[None, :]], axis=0))
        cols_v += [bf_, bs_, gm_, be_]
    W = np.ascontiguousarray(np.concatenate(blocks_w, axis=1),
                             dtype=np.float32)             # [128,1024]
    WC = np.ascontiguousarray(np.concatenate(blocks_wc, axis=1)).astype(bfd)
    V = np.ascontiguousarray(np.stack(cols_v, axis=1), dtype=np.float32)

    in_maps = []
    for cid in range(N_CORES):
        rows = slice(cid * NL, (cid + 1) * NL)
        cx = centers[rows].T                               # [2, NL]
        ca = np.concatenate([cx, np.ones((1, NL), np.float32)], axis=0)
        in_maps.append({
            "xT": np.ascontiguousarray(gnn_in[rows].T, dtype=np.float32),
            "cA": np.ascontiguousarray(ca).astype(bfd),
            "W": W, "WC": WC, "V": V,
        })
    return in_maps


def _canonical_edge_index():
    i, j = np.meshgrid(np.arange(A), np.arange(A), indexing="ij")
    mask = i != j
    li, lj = i[mask], j[mask]
    offs = (np.arange(B) * A)[:, None]
    rows = (li[None, :] + offs).reshape(-1)
    cols = (lj[None, :] + offs).reshape(-1)
    return np.stack([rows, cols])


def _numpy_fallback(gnn_in, centers, edge_index, params):
    """Generic (slow) host implementation for non-canonical edge_index."""
    row, col = np.asarray(edge_index[0]), np.asarray(edge_index[1])
    eattr = centers[col] - centers[row]
    x = gnn_in

    def softplus(z):
        return np.maximum(z, 0.0) + np.log1p(np.exp(-np.abs(z)))

    def cgconv(x, wf, bf, ws, bs, gm, be):
        z = np.concatenate([x[col], x[row], eattr], axis=-1)
        mf = 1.0 / (1.0 + np.exp(-(z @ wf.T + bf)))
        m = mf * softplus(z @ ws.T + bs)
        agg = np.zeros_like(x)
        np.add.at(agg, col, m)
        mean = agg.mean(axis=0)
        var = agg.var(axis=0)
        bn = (agg - mean) / np.sqrt(var + EPS) * gm + be
        return bn + x

    x = np.maximum(cgconv(x, *params[0]), 0.0)
    x = np.maximum(cgconv(x, *params[1]), 0.0)
    return x.astype(np.float32)


# --------------------------------------------------------------------------
# entry point
# --------------------------------------------------------------------------

def kernel(gnn_in, centers, edge_index, wf1, bf1, ws1, bs1, g1, be1,
           wf2, bf2, ws2, bs2, g2, be2):
    gnn_in = np.asarray(gnn_in, dtype=np.float32)
    centers = np.asarray(centers, dtype=np.float32)
    args = [np.asarray(a, dtype=np.float32)
            for a in (wf1, bf1, ws1, bs1, g1, be1,
                      wf2, bf2, ws2, bs2, g2, be2)]

    ei = np.asarray(edge_index)
    if ei.shape != (2, B * A * (A - 1)) or \
            not np.array_equal(ei, _canonical_edge_index()):
        return _numpy_fallback(gnn_in, centers, ei,
                               (tuple(args[0:6]), tuple(args[6:12])))

    from concourse import bass_utils
    nc = get_nc()
    in_maps = prep_in_maps(gnn_in, centers, *args)
    res = bass_utils.run_bass_kernel_spmd(nc, in_maps,
                                          core_ids=list(range(N_CORES)))
    out = np.empty((N, H), dtype=np.float32)
    for cid in range(N_CORES):
        out[cid * NL:(cid + 1) * NL] = res.results[cid]["outT"].T
    return out
